# revision 1
# baseline (speedup 1.0000x reference)
"""Distributed causal multi-head attention for Trainium2 (8 NeuronCores).

Reference computes, for x [2, 2048, 1024]:
    qkv = x @ W_qkv + b_qkv ; split into q,k,v heads (16 heads, d_k=64)
    causal softmax attention per head
    out = ctx @ W_o + b_o

Sharding (data + head parallel): core c handles batch b=c//4 and heads
H = [4g..4g+3] with g=c%4.  Each core:
  - computes q^T,k^T ([dk, T] layout, head-pairs packed into 128 partitions)
    and v ([T, dk] natural layout, augmented with a ones column so the
    attention-weights matmul also produces softmax denominators),
  - computes its heads' causal T x T attention to get ctx^T [256, T],
  - AllGathers ctx^T within its 4-core batch group -> [1024, T],
  - computes a disjoint 256-column slice of the output projection.
Host-side: shard prep (transpose/slice/bf16-cast) and a pure concat of the
8 output column-slices.  All FLOPs (matmuls, softmax, reductions) on device.
"""

import numpy as np
import ml_dtypes

import concourse.bass as bass
import concourse.mybir as mybir
import concourse.tile as tile
from concourse import bacc
from concourse import bass_utils

BF16 = mybir.dt.bfloat16
F32 = mybir.dt.float32
AF = mybir.ActivationFunctionType

T = 2048
D = 1024
NH = 16
HPC = 4  # heads per core
DK = 64
NCORES = 8
TQ = 512  # q-chunk (free dim of logits^T tiles)
NQC = T // TQ  # 4
NKT = T // 128  # 16 k-tiles
NDT = D // 128  # 8 d-tiles
NTT = T // 128  # 16 t-tiles
VW = DK + 1  # 65: v columns per head incl. ones column
SCALE = 1.0 / 8.0  # 1/sqrt(DK)

TRACE = False  # set True (with profile shim installed) to capture HW profile
LAST_RESULT = {}

_cache = {}


def _build():
    nc = bacc.Bacc("TRN2", target_bir_lowering=False, debug=False,
                   num_devices=NCORES)

    xt = nc.declare_dram_parameter("xt", [D, T], BF16, False)
    wq = nc.declare_dram_parameter("wq", [D, 256], BF16, False)
    wk = nc.declare_dram_parameter("wk", [D, 256], BF16, False)
    wv = nc.declare_dram_parameter("wv", [D, HPC * VW], BF16, False)
    wo = nc.declare_dram_parameter("wo", [D, 256], BF16, False)
    bq = nc.declare_dram_parameter("bq", [128, 2], F32, False)
    bk = nc.declare_dram_parameter("bk", [128, 2], F32, False)
    bv = nc.declare_dram_parameter("bv", [128, HPC * VW], F32, False)
    bo = nc.declare_dram_parameter("bo", [128, 256], F32, False)
    masks = nc.declare_dram_parameter("masks", [128, 4 * TQ], BF16, False)
    out = nc.declare_dram_parameter("out", [T, 256], F32, True)

    cc_in = [nc.dram_tensor(f"cc_in{h}", [DK, T], BF16) for h in range(HPC)]
    cc_out = [nc.dram_tensor(f"cc_out{h}", [4 * DK, T], BF16) for h in range(HPC)]

    with tile.TileContext(nc) as tc, tc.tile_pool(name="pers", bufs=1) as pers:
        # ---------------- persistent SBUF ----------------
        xt_sb = pers.tile([128, NDT * T], BF16, tag="xt_sb", name="xt_sb")
        wq_sb = pers.tile([128, NDT * 256], BF16, tag="wq_sb", name="wq_sb")
        wk_sb = pers.tile([128, NDT * 256], BF16, tag="wk_sb", name="wk_sb")
        wv_sb = pers.tile([128, NDT * HPC * VW], BF16, tag="wv_sb", name="wv_sb")
        wo_sb = pers.tile([128, NDT * 256], BF16, tag="wo_sb", name="wo_sb")
        bq_sb = pers.tile([128, 2], F32, tag="bq_sb", name="bq_sb")
        bk_sb = pers.tile([128, 2], F32, tag="bk_sb", name="bk_sb")
        bv_sb = pers.tile([128, HPC * VW], F32, tag="bv_sb", name="bv_sb")
        bo_sb = pers.tile([128, 256], F32, tag="bo_sb", name="bo_sb")
        mask_sb = pers.tile([128, 4 * TQ], BF16, tag="mask_sb", name="mask_sb")
        qT_sb = pers.tile([128, 2 * T], BF16, tag="qT_sb", name="qT_sb")
        kT_sb = pers.tile([128, 2 * T], BF16, tag="kT_sb", name="kT_sb")
        v_sb = pers.tile([128, NTT * HPC * VW], BF16, tag="v_sb", name="v_sb")
        ctxg_sb = pers.tile([128, NDT * T], BF16, tag="ctxg_sb", name="ctxg_sb")

        for d in range(NDT):
            nc.sync.dma_start(wq_sb[:, d * 256:(d + 1) * 256],
                              wq[128 * d:128 * (d + 1), :])
            nc.sync.dma_start(wk_sb[:, d * 256:(d + 1) * 256],
                              wk[128 * d:128 * (d + 1), :])
            nc.sync.dma_start(wv_sb[:, d * HPC * VW:(d + 1) * HPC * VW],
                              wv[128 * d:128 * (d + 1), :])
            nc.sync.dma_start(wo_sb[:, d * 256:(d + 1) * 256],
                              wo[128 * d:128 * (d + 1), :])
        for qc in range(NQC):
            for d in range(NDT):
                nc.sync.dma_start(
                    xt_sb[:, d * T + qc * TQ:d * T + (qc + 1) * TQ],
                    xt[128 * d:128 * (d + 1), qc * TQ:(qc + 1) * TQ])
        nc.sync.dma_start(bq_sb[:], bq[:])
        nc.sync.dma_start(bk_sb[:], bk[:])
        nc.sync.dma_start(bv_sb[:], bv[:])
        nc.sync.dma_start(bo_sb[:], bo[:])
        nc.sync.dma_start(mask_sb[:], masks[:])

        with (
            tc.tile_pool(name="pp", space="PSUM", bufs=2) as pp,
            tc.tile_pool(name="sp", space="SBUF", bufs=2) as sp,
        ):
            # ---------------- QKV projections (chunked emitters) ----------
            # QKV work is interleaved between attention items below so the
            # scalar engine starts exp work early and QKV matmuls fill PE
            # gaps throughout the ACT-paced attention phase.
            # ---------------- QKV projections ----------------
            # round-robin QKV psum tiles over all tags (the attention-phase
            # tags are idle during QKV) for deep buffering
            qkv_tags = ["lgX", "lgX", "lgY", "ctxX", "ctxY"]
            qkv_bufs = {"lgX": 2, "lgY": 1, "ctxX": 1, "ctxY": 1}
            qkv_ctr = [0]

            def qkv_tag():
                t = qkv_tags[qkv_ctr[0] % len(qkv_tags)]
                qkv_ctr[0] += 1
                return t

            def emit_qk(p):
                for qc in range(NQC):
                    _t = qkv_tag()
                    psq = pp.tile([128, TQ], F32, tag=_t, bufs=qkv_bufs[_t],
                                  name=f"psq_{p}_{qc}")
                    for d in range(NDT):
                        nc.tensor.matmul(
                            psq[:],
                            lhsT=wq_sb[:, d * 256 + 128 * p:
                                       d * 256 + 128 * p + 128],
                            rhs=xt_sb[:, d * T + qc * TQ:d * T + (qc + 1) * TQ],
                            start=(d == 0), stop=(d == NDT - 1))
                    nc.vector.tensor_scalar_add(
                        qT_sb[:, p * T + qc * TQ:p * T + (qc + 1) * TQ],
                        psq[:], bq_sb[:, p:p + 1])
                    _t = qkv_tag()
                    psk = pp.tile([128, TQ], F32, tag=_t, bufs=qkv_bufs[_t],
                                  name=f"psk_{p}_{qc}")
                    for d in range(NDT):
                        nc.tensor.matmul(
                            psk[:],
                            lhsT=wk_sb[:, d * 256 + 128 * p:
                                       d * 256 + 128 * p + 128],
                            rhs=xt_sb[:, d * T + qc * TQ:d * T + (qc + 1) * TQ],
                            start=(d == 0), stop=(d == NDT - 1))
                    nc.vector.tensor_scalar_add(
                        kT_sb[:, p * T + qc * TQ:p * T + (qc + 1) * TQ],
                        psk[:], bk_sb[:, p:p + 1])

            def emit_v():
                W = HPC * VW
                for tt in range(NTT):
                    _t = qkv_tag()
                    psv = pp.tile([128, W], F32, tag=_t, bufs=qkv_bufs[_t],
                                  name=f"psv_{tt}")
                    for d in range(NDT):
                        nc.tensor.matmul(
                            psv[:],
                            lhsT=xt_sb[:, d * T + tt * 128:d * T + (tt + 1) * 128],
                            rhs=wv_sb[:, d * W:(d + 1) * W],
                            start=(d == 0), stop=(d == NDT - 1))
                    nc.vector.tensor_add(v_sb[:, tt * W:(tt + 1) * W],
                                         psv[:], bv_sb[:])

            # ---------------- attention: two interleaved head chains ------
            # chain X = heads {0,2} (partition rows 0-63), chain Y = heads
            # {1,3} (rows 64-127): their K=64 logits matmuls occupy disjoint
            # PE row-groups and run concurrently; two chains keep PE busy
            # while the other chain waits on exp.
            def emit_ag(h):
                # per-head all-gather within the 4-core batch group, issued as
                # soon as this head's ctx^T is written -> overlaps the
                # remaining heads' compute
                nc.gpsimd.collective_compute(
                    "AllGather",
                    mybir.AluOpType.bypass,
                    replica_groups=[[0, 1, 2, 3], [4, 5, 6, 7]],
                    ins=[cc_in[h].ap().opt()],
                    outs=[cc_out[h].ap().opt()],
                )
                for j in range(2):
                    ct = 2 * h + j
                    nc.sync.dma_start(ctxg_sb[:, ct * T:(ct + 1) * T],
                                      cc_out[h][128 * j:128 * (j + 1), :])

            # ---------------- output projection (four stages) -------------
            # stage s consumes AG(s)'s two c-tiles as soon as that gather
            # lands; partials accumulate in SBUF.  Stage 3 adds bias + stores.
            acc_sb = pers.tile([128, NTT * 256], F32, tag="acc_sb",
                               name="acc_sb")

            def emit_proj_stage(s):
                for tt in range(NTT):
                    po = pp.tile([128, 256], F32,
                                 tag=("ctxX" if tt % 2 == 0 else "ctxY"), bufs=1,
                                 name=f"po_{s}_{tt}")
                    for k in range(2):
                        ct = 2 * s + k
                        nc.tensor.matmul(
                            po[:],
                            lhsT=ctxg_sb[:, ct * T + tt * 128:
                                         ct * T + (tt + 1) * 128],
                            rhs=wo_sb[:, ct * 256:(ct + 1) * 256],
                            start=(k == 0), stop=(k == 1))
                    a = acc_sb[:, tt * 256:(tt + 1) * 256]
                    if s == 0:
                        nc.vector.tensor_add(a, po[:], bo_sb[:])
                    elif s < 3:
                        nc.vector.tensor_add(a, po[:], a)
                    else:
                        o_sb = sp.tile([128, 256], F32, tag="o_sb", bufs=3,
                                       name=f"o_{tt}")
                        nc.vector.tensor_add(o_sb[:], po[:], a)
                        nc.sync.dma_start(out[128 * tt:128 * (tt + 1), :],
                                          o_sb[:])

            def emit_attn_pair(hx, hy, qc):
                # heads hx (partition rows 0-63) and hy (rows 64-127) advance
                # in lockstep; their K=64 logits matmuls are emitted adjacent
                # so the PE packs them into disjoint row-groups.
                nkt = 4 * qc + 4
                ctxs = {}
                lgs = {}
                exs = {}
                for grp in range(nkt // 2):
                    for h, cn in ((hx, "X"), (hy, "Y")):
                        if grp == 0:
                            ctxs[cn] = pp.tile([VW, TQ], F32, tag=f"ctx{cn}",
                                               bufs=1, name=f"ctx_{h}_{qc}")
                        lgs[cn] = pp.tile([128, 2 * TQ], F32, tag=f"lg{cn}",
                                          bufs=(2 if cn == "X" else 1),
                                          name=f"lg_{h}_{qc}_{grp}")
                        exs[cn] = sp.tile([128, 2 * TQ], BF16, tag=f"ex{cn}",
                                          bufs=5, name=f"ex_{h}_{qc}_{grp}")
                    for j in range(2):
                        kt = 2 * grp + j
                        for h, cn in ((hx, "X"), (hy, "Y")):
                            p, half = h // 2, h % 2
                            r0 = DK * half
                            nc.tensor.matmul(
                                lgs[cn][:, j * TQ:(j + 1) * TQ],
                                lhsT=kT_sb[r0:r0 + DK,
                                           p * T + kt * 128:p * T + (kt + 1) * 128],
                                rhs=qT_sb[r0:r0 + DK,
                                          p * T + qc * TQ:p * T + (qc + 1) * TQ],
                                start=True, stop=True)
                    for h, cn in ((hx, "X"), (hy, "Y")):
                        nc.scalar.activation(exs[cn][:], lgs[cn][:], AF.Exp,
                                             scale=SCALE)
                    for h, cn in ((hx, "X"), (hy, "Y")):
                        for j in range(2):
                            kt = 2 * grp + j
                            if kt >= 4 * qc:
                                r = kt - 4 * qc
                                nc.vector.tensor_mul(
                                    exs[cn][:, j * TQ:(j + 1) * TQ],
                                    exs[cn][:, j * TQ:(j + 1) * TQ],
                                    mask_sb[:, r * TQ:(r + 1) * TQ])
                            nc.tensor.matmul(
                                ctxs[cn][:],
                                lhsT=v_sb[:, kt * HPC * VW + VW * h:
                                          kt * HPC * VW + VW * h + VW],
                                rhs=exs[cn][:, j * TQ:(j + 1) * TQ],
                                start=(kt == 0), stop=(kt == nkt - 1))
                for h, cn in ((hx, "X"), (hy, "Y")):
                    ctx = ctxs[cn]
                    # drain psum fast (DVE only) so the attention pipeline
                    # never waits on the division chain -- the division below
                    # is SBUF-only, so a Pool queue blocked on an in-flight
                    # collective cannot hold a psum bank hostage.
                    ctxu = sp.tile([DK, TQ], F32, tag=f"ctxu{cn}", bufs=4,
                                   name=f"ctxu_{h}_{qc}")
                    nc.vector.tensor_copy(ctxu[:], ctx[0:DK, :])
                    dn = sp.tile([1, TQ], F32, tag=f"dn{cn}", bufs=4,
                                 name=f"dn_{h}_{qc}")
                    nc.vector.tensor_copy(dn[:], ctx[DK:DK + 1, :])
                    rc = sp.tile([1, TQ], F32, tag=f"rc{cn}", bufs=2,
                                 name=f"rc_{h}_{qc}")
                    nc.vector.reciprocal_approx_fast(rc[:], dn[:])
                    rcb = sp.tile([DK, TQ], F32, tag=f"rcb{cn}", bufs=2,
                                  name=f"rcb_{h}_{qc}")
                    nc.gpsimd.partition_broadcast(rcb[:], rc[:])
                    ctxd = sp.tile([DK, TQ], BF16, tag=f"ctxd{cn}", bufs=2,
                                   name=f"ctxd_{h}_{qc}")
                    nc.vector.tensor_mul(ctxd[:], ctxu[:], rcb[:])
                    nc.sync.dma_start(cc_in[h][:, qc * TQ:(qc + 1) * TQ],
                                      ctxd[:])

            emit_qk(0)
            emit_v()
            emit_qk(1)
            for qc in range(NQC):
                emit_attn_pair(0, 1, qc)
            emit_ag(0)
            emit_ag(1)
            for qc in range(NQC):
                emit_attn_pair(2, 3, qc)
            # AG(0)/AG(1) have landed; their half of the output projection
            # fills PE while heads 2,3 wind down
            emit_proj_stage(0)
            emit_proj_stage(1)
            emit_ag(2)
            emit_ag(3)
            emit_proj_stage(2)
            emit_proj_stage(3)


    nc.compile()
    return nc


def _masks_np():
    jj = np.arange(128)[:, None]
    ii = np.arange(TQ)[None, :]
    m = np.zeros((128, 4 * TQ), np.float32)
    for r in range(4):
        m[:, r * TQ:(r + 1) * TQ] = (jj + 128 * r <= ii)
    return m.astype(ml_dtypes.bfloat16)


def _wo_reorder(Wo, g):
    # device c-tile slot ct=2h+j must hold W_o rows for heads (8j+h, 8j+4+h)
    blocks = []
    for h in range(HPC):
        for j in range(2):
            for r in (2 * j, 2 * j + 1):
                gh = 4 * r + h
                blocks.append(Wo[gh * DK:(gh + 1) * DK, 256 * g:256 * (g + 1)])
    return np.ascontiguousarray(np.concatenate(blocks, axis=0))


def _shard_inputs(x, Wqkv, bqkv, Wo, bo_v):
    bf = ml_dtypes.bfloat16
    masks = _masks_np()
    in_maps = []
    for c in range(NCORES):
        b, g = c // 4, c % 4
        h0 = 4 * g
        q0 = h0 * DK
        wv = np.zeros((D, HPC * VW), np.float32)
        bv = np.zeros((HPC * VW,), np.float32)
        for j in range(HPC):
            wv[:, VW * j:VW * j + DK] = Wqkv[:, 2 * D + (h0 + j) * DK:
                                             2 * D + (h0 + j + 1) * DK]
            bv[VW * j:VW * j + DK] = bqkv[2 * D + (h0 + j) * DK:
                                          2 * D + (h0 + j + 1) * DK]
            bv[VW * j + DK] = 1.0
        in_maps.append({
            "xt": np.ascontiguousarray(x[b].T).astype(bf),
            "wq": np.ascontiguousarray(Wqkv[:, q0:q0 + 256]).astype(bf),
            "wk": np.ascontiguousarray(Wqkv[:, D + q0:D + q0 + 256]).astype(bf),
            "wv": wv.astype(bf),
            "wo": _wo_reorder(Wo, g).astype(bf),
            "bq": np.stack([bqkv[q0:q0 + 128], bqkv[q0 + 128:q0 + 256]],
                           axis=1).astype(np.float32).copy(),
            "bk": np.stack([bqkv[D + q0:D + q0 + 128],
                            bqkv[D + q0 + 128:D + q0 + 256]],
                           axis=1).astype(np.float32).copy(),
            "bv": np.ascontiguousarray(
                np.broadcast_to(bv, (128, HPC * VW))).astype(np.float32),
            "bo": np.ascontiguousarray(
                np.broadcast_to(bo_v[256 * g:256 * (g + 1)], (128, 256))
            ).astype(np.float32),
            "masks": masks,
        })
    return in_maps


def kernel(**inputs):
    x = np.asarray(inputs["x"], np.float32)
    Wqkv = np.asarray(inputs["W_qkv"], np.float32)
    bqkv = np.asarray(inputs["b_qkv"], np.float32)
    Wo = np.asarray(inputs["W_o"], np.float32)
    bo_v = np.asarray(inputs["b_o"], np.float32)

    if "nc" not in _cache:
        _cache["nc"] = _build()
    nc = _cache["nc"]

    in_maps = _shard_inputs(x, Wqkv, bqkv, Wo, bo_v)
    res = bass_utils.run_bass_kernel_spmd(
        nc, in_maps, core_ids=list(range(NCORES)), trace=TRACE)
    LAST_RESULT["exec_time_ns"] = res.exec_time_ns
    LAST_RESULT["res"] = res

    out = np.empty((2, T, D), np.float32)
    for c in range(NCORES):
        out[c // 4, :, 256 * (c % 4):256 * (c % 4 + 1)] = res.results[c]["out"]
    return out

